# revision 37
# baseline (speedup 1.0000x reference)
"""Sobel filter Trainium2 Bass kernel (v2).

Problem: img [32, 3, 512, 512] f32, kx/ky [1, 3, 3, 3] f32. Output
[32, 1, 512, 512] f32:
    Gx = valid_conv3x3(img, kx), Gy = valid_conv3x3(img, ky)  -> [N,1,510,510]
    out = sqrt(Gx^2 + Gy^2) edge-padded by 1 back to [N,1,512,512]

Pure data parallel over 8 NeuronCores, 4 images per core, fp16 on device
(host casts / transposes; rel err ~1e-3 vs the 2e-2 gate). The problem is
memory-regime: 6.3 MB load + 2.1 MB store per core ≈ 24 us at peak HBM.

v2 layout (vs the 70 us v1): supertiles span ALL 4 images (ops are
[128, 2048]; 4KB DMA descriptors), and the arithmetic is restructured so
every engine stays under ~27 us:

  * All 12 big loads ([128, 4, 512] fp16 per (super, channel)) issue at
    t=0 across three DGE rings (sync/tensor/gpsimd), so the 16 DMA
    engines stream continuously instead of waiting on serialized issue
    (DMA_DIRECT2D costs ~0.7 us of ring-queue time per 128-desc load).
  * Channel presum T = c0+c1+c2: two DVE 2x tensor-adds.
  * Shared pair-add trick: u = T[x]+T[x+1] feeds BOTH x-convs:
      Tv = u[x]+u[x+1] = T * [1,2,1]x   (DVE, for the [1,2,1]-x group)
      D  = u[x]-u[x+1] = T[x]-T[x+2]    (GPSIMD, for the [1,0,-1]-x group)
    so each group needs ONE banded y-conv matmul per image (8 MM/super,
    PE ~23 us) and GPSIMD carries one full pass (it has no PSUM port and
    can't help with drains anyway).
  * Cross-image garbage columns from the flat u/Tv/D ops land in columns
    the per-image matmul movers never read.
  * PSUM: pair tiles S1/S2 [M, 2, 512] f32 (2 banks) x2 bufs = 8 banks.
  * Drain: ACT square per (pair, group) -> s1/s2 [M, 4, 512] fp16; then
    per super DVE 2x combine ss = s1+s2, ACT edge-dup (strided 2-col
    copy), ACT sqrt full width, one [M, 4*512] store (4KB descriptors),
    alternating scalar/sync rings.
  * A custom DVE op (registered at runtime into dve_ops.OPS, as the
    authoring guide prescribes) fuses ss = sq(S2_psum) + s1_sbuf for some
    supers to shift drain work ACT->DVE; the split is a tunable knob.
  * Mini tile: rows 504-511 of all 4 images packed [32, 512] in the
    partition dim (8 rows x 4 images), same u/Tv/D + 2-MM structure with
    block-diagonal stationaries, bottom-pad row baked into the bands.

Fallback for non-Sobel-shaped kernels: generic fp32r path (18 banded
matmuls per tile, full 2D conv on PE).
"""

import os

import numpy as np

N_CORES = 8
N_FULL = 32          # full batch
N_PER_CORE = N_FULL // N_CORES
H = W = 512
NW = 510             # valid output columns
WF = N_PER_CORE * W  # 2048 flat columns per supertile
WPAD = WF + 16       # flat tiles padded so shifted ops keep even widths

# supertile rows: 4 supers + mini (valid-conv row v reads input rows v..v+2;
# out row o>=1 is v_{o-1}; t0 col 0 dups v0 = top pad)
SUP_Y0 = [0, 126, 252, 378]   # input row start
SUP_O0 = [0, 127, 253, 379]   # output row start
SUP_M = [127, 126, 126, 126]  # output rows
MINI_Y0 = 504                 # mini input rows 504..511 (8 per image)
MINI_O0 = 505                 # mini output rows 505..511 (7 per image)
MINI_KI = 8
MINI_M = 28                   # 7 out rows x 4 images

# generic-path constants (fp32r fallback)
TILE_K = 128
TILE_M = 126
N_TILES = 4
G_MINI_K = 8 * N_PER_CORE
G_MINI_M = 6 * N_PER_CORE

_CACHE: dict = {}
LAST_RESULTS = None  # BassKernelResults of the most recent run (for test.py)


# ---------------------------------------------------------------------------
# Custom DVE op: ss = sq(in0) + in1 (in0 may be PSUM). Registered at runtime
# following the dve_ops authoring guide (define DveOp, append to OPS).
# ---------------------------------------------------------------------------


def _register_sq_plus():
    import concourse.dve_ops as dve_ops
    from concourse.dve_ops import DveOp, OPS, _SUB_OPCODE_FOR_NAME
    from concourse.dve_spec import Spec, Src0, Src1, sq
    from concourse.dve_spec import lower as dve_lower
    from concourse.dve_uop import DveOpSpec

    name = "SQ_PLUS_ANT"
    if name in _SUB_OPCODE_FOR_NAME:
        return next(op for op in OPS if op.name == name)

    spec = Spec(
        body=sq(Src0) + Src1,
        reference=lambda in0, in1, s0, s1, imm2: (
            in0.astype(np.float32) ** 2 + in1
        ).astype(np.float32),
    )
    row = max(_SUB_OPCODE_FOR_NAME.values()) + 1
    assert row < 0x20
    _SUB_OPCODE_FOR_NAME[name] = row
    shas = {}
    for ver in ("v3", "v4"):
        s = DveOpSpec(name=name, opcode=row, uops=dve_lower(spec, ver=ver),
                      rd1_en=True)
        shas[ver] = s.sha(ver)
    op = DveOp(name, spec, subdim=False, uops_sha=shas)
    OPS.append(op)
    dve_ops.CUSTOM_DVE_SPECS[name] = spec
    return op


# ---------------------------------------------------------------------------
# Fast path: Sobel-shaped kernels (shared channel weights, one group with
# antisymmetric distance-2 x-taps, one with binomial x-taps).
# ---------------------------------------------------------------------------


def _band_std(b):
    """[128, 126]: col m computes valid row v=m (taps at rows m..m+2)."""
    A = np.zeros((128, 126), np.float64)
    m = np.arange(126)
    for dy in range(3):
        A[m + dy, m] = b[dy]
    return A


def _band_t0(b):
    """[128, 127]: col 0 duplicates v0 (top edge pad), col m+1 = v_m."""
    A = np.zeros((128, 127), np.float64)
    A[0:3, 0] = b
    m = np.arange(126)
    for dy in range(3):
        A[m + dy, m + 1] = b[dy]
    return A


def _band_mini(b):
    """[32, 28] block matrix: input partition 8n+j (image n, row 504+j),
    output col 7n+r (out row 505+r of image n; n-major so each image's
    mini store is one contiguous 7-partition DMA). Col r<=5 computes
    v504+r, col r=6 duplicates v509 (bottom edge pad)."""
    A = np.zeros((32, MINI_M), np.float64)
    for n in range(N_PER_CORE):
        for r in range(6):
            for dy in range(3):
                A[MINI_KI * n + r + dy, 7 * n + r] = b[dy]
        for dy in range(3):
            A[MINI_KI * n + 5 + dy, 7 * n + 6] = b[dy]
    return A


def _try_fast(kx, ky):
    """Detect k[g,c,dy,dx] = r_c * b_g[dy] * g_g[dx] with unit channel
    ratios, one group with x-taps [a, 0, -a], the other with x-taps
    prop. to [1, 2, 1]. Returns (stat, statm, key) or None. Bands are
    pre-scaled so that
      Gx = bandX (x) conv_y D,   D  = T[x] - T[x+2]
      Gy = bandY (x) conv_y Tv,  Tv = T[x] + 2 T[x+1] + T[x+2]
    """
    k = np.stack([np.asarray(kx, np.float64)[0], np.asarray(ky, np.float64)[0]])
    scale = np.abs(k).max()
    if scale == 0:
        return None
    tol = 1e-6 * scale

    # all channels identical (torch expand -> unit ratios)
    for c in range(1, 3):
        if np.abs(k[:, c] - k[:, 0]).max() > tol:
            return None
    base = k[:, 0]  # [2, dy, dx]

    bands = [None, None]  # [D-group band, Tv-group band]
    for g in range(2):
        B = base[g]  # [dy, dx]
        norms = np.sqrt((B * B).sum(axis=0))
        if norms[0] > tol and norms[2] > tol and norms[1] <= tol:
            # candidate antisymmetric distance-2 group: B[:,2] == -B[:,0]
            if np.abs(B[:, 2] + B[:, 0]).max() > tol:
                return None
            if bands[0] is not None:
                return None
            bands[0] = B[:, 0].copy()
        elif norms[1] > tol:
            # candidate [lam, 1, lam]*center group with lam == 0.5
            c = B[:, 1]
            lam = 0.5
            if (np.abs(B[:, 0] - lam * c).max() > tol
                    or np.abs(B[:, 2] - lam * c).max() > tol):
                return None
            if bands[1] is not None:
                return None
            bands[1] = (B[:, 1] * lam).copy()  # Tv = T*[1,2,1]; band carries 1/2
        else:
            return None
    if bands[0] is None or bands[1] is None:
        return None
    # which group is Gx (the D group)? bands[0] came from whichever g matched;
    # track group order so squares sum correctly (order is irrelevant for
    # Gx^2+Gy^2, so no need to track).

    stat = np.zeros((128, 4 * 128), np.float64)
    statm = np.zeros((32, 2 * MINI_M), np.float64)
    for i, b in enumerate(bands):
        stat[:, 128 * i: 128 * i + 127] = _band_t0(b)
        stat[:, 128 * (2 + i): 128 * (2 + i) + 126] = _band_std(b)
        statm[:, MINI_M * i: MINI_M * (i + 1)] = _band_mini(b)
    key = ("fast2", tuple(np.round(np.concatenate(bands), 12)))
    return (
        np.ascontiguousarray(stat.astype(np.float16)),
        np.ascontiguousarray(statm.astype(np.float16)),
        key,
    )


def _sobel_body_fast(tc, out, img, stat_dram, statm_dram, sq_plus):
    """img dram [3, 512, 4, 512] fp16 (c, y, n, x): per-(c,y) rows are 4KB
    contiguous DMA descriptors. out dram [512, 4, 512] fp16."""
    import concourse.mybir as mybir

    nc = tc.nc
    f32 = mybir.dt.float32
    f16 = mybir.dt.float16

    img_ny = img.rearrange("c y n x -> c n y x")  # [3, 4, 512, 512]

    # which supers drain via the fused DVE op (sq2+combine) instead of
    # ACT-square + DVE-add; knob balances ACT vs DVE busy time.
    n_dve_drain = int(os.environ.get("SOBEL_DVE_DRAIN", "2"))

    with (
        tc.tile_pool(name="const", bufs=1) as const_pool,
        tc.tile_pool(name="imgs", bufs=4) as img_pool,
        tc.tile_pool(name="work", bufs=2) as work_pool,
        tc.tile_pool(name="psum", bufs=2, space="PSUM") as psum_pool,
    ):
        stat_sb = const_pool.tile([128, 4 * 128], f16)
        statm_sb = const_pool.tile([32, 2 * MINI_M], f16)

        # --- all loads up front, split across the two HWDGE rings --------
        # (only SP and Activation issue hardware-DGE DMAs). Each ring has
        # ~4 outstanding-DMA credits, so order = priority: supertile 0's
        # three channels go first on BOTH rings so compute starts ~10.5 us
        # instead of ~14. Stores ride sync exclusively: scalar-ring (q10)
        # DMAs were measured to dump a whole store's descriptors on ONE
        # DMA engine (~21 GB/s).
        cts = [
            [
                img_pool.tile([128, WF], f16, tag=f"c{c}", name=f"c{c}_{t}",
                              bufs=4)
                for c in range(3)
            ]
            for t in range(4)
        ]
        mit = img_pool.tile([32, 3, W], f16, tag="mit", name="mit", bufs=1)

        def _load(t, c, ring, half=None):
            ct4 = cts[t][c].rearrange("k (n x) -> k n x", n=N_PER_CORE)
            if half is None:
                ring.dma_start(out=ct4, in_=img[c, SUP_Y0[t]: SUP_Y0[t] + 128])
            else:
                n0 = 2 * half
                ring.dma_start(
                    out=ct4[:, n0: n0 + 2],
                    in_=img[c, SUP_Y0[t]: SUP_Y0[t] + 128, n0: n0 + 2],
                )

        _load(0, 0, nc.sync)
        _load(0, 2, nc.scalar)
        _load(0, 1, nc.sync)
        for c in range(3):
            nc.scalar.dma_start(
                out=mit[:, c], in_=img_ny[c, :, MINI_Y0: MINI_Y0 + MINI_KI]
            )
        _load(1, 0, nc.sync)
        _load(1, 1, nc.scalar)
        _load(1, 2, nc.scalar)
        nc.scalar.dma_start(out=stat_sb, in_=stat_dram)
        nc.scalar.dma_start(out=statm_sb, in_=statm_dram)
        _load(2, 0, nc.sync)
        _load(2, 1, nc.sync)
        _load(2, 2, nc.scalar)
        _load(3, 0, nc.sync)
        _load(3, 1, nc.scalar)
        _load(3, 2, nc.scalar)

        store_ctr = [0]

        def xprep(tag, K, Wv, Wp, c0, c1, c2, bufs=1):
            """presum + u + Tv + D, all DVE 2x passes. (GPSIMD was measured
            to be unusable for stream work: each op costs ~4 us AND slows
            any concurrently-running DVE op ~4x.) Tiles padded to Wp; ops
            even-width Wv, reading at most one uninitialized column (never
            consumed by the per-image matmul movers).

            bufs=1 on T/u forces the scheduler to run the supertile chains
            in emission order (with free buffers it reordered the
            independent chains [0,3,2,1], head-of-line blocking the DVE
            queue ~6 us waiting on t3's loads)."""
            T = work_pool.tile([K, Wp], f16, tag=f"T{tag}", name=f"T{tag}",
                               bufs=bufs)
            nc.vector.tensor_add(T[:, 0:Wv], c0, c1)
            nc.vector.tensor_add(T[:, 0:Wv], T[:, 0:Wv], c2)
            u = work_pool.tile([K, Wp], f16, tag=f"u{tag}", name=f"u{tag}",
                               bufs=bufs)
            nc.vector.tensor_add(u[:, 0:Wv], T[:, 0:Wv], T[:, 1: 1 + Wv])
            Tv = work_pool.tile([K, Wp], f16, tag=f"Tv{tag}", name=f"Tv{tag}",
                                bufs=max(2, bufs))
            nc.vector.tensor_add(Tv[:, 0:Wv], u[:, 0:Wv], u[:, 1: 1 + Wv])
            D = work_pool.tile([K, Wp], f16, tag=f"D{tag}", name=f"D{tag}",
                               bufs=max(2, bufs))
            nc.vector.tensor_sub(D[:, 0:Wv], u[:, 0:Wv], u[:, 1: 1 + Wv])
            return Tv, D

        def stage_a(t):
            """x-prep + MMs + per-pair ACT square drains for supertile t."""
            y0, o0, M = SUP_Y0[t], SUP_O0[t], SUP_M[t]
            xoff, yoff = (0, 128) if t == 0 else (256, 384)
            c0, c1, c2 = (ct[:, 0:WF] for ct in cts[t])
            Tv, D = xprep("", 128, WF, WPAD, c0, c1, c2)
            srcs = [(Tv, D, 0), (Tv, D, 1024)]

            s1 = work_pool.tile([M, N_PER_CORE, W], f16, tag="s1",
                                name=f"s1_{t}", bufs=3)
            s2 = work_pool.tile([M, N_PER_CORE, W], f16, tag="s2",
                                name=f"s2_{t}", bufs=3)
            ss = work_pool.tile([M, N_PER_CORE, W], f16, tag="ss",
                                name=f"ss_{t}", bufs=3)
            S2s = []
            for p in range(2):
                Tvp, Dp, base = srcs[p]
                S1 = psum_pool.tile([M, 2, W], f32, tag="S1", name=f"S1_{t}{p}")
                for i in range(2):
                    nc.tensor.matmul(
                        S1[:, i, 0:NW],
                        stat_sb[:, xoff: xoff + M],
                        Dp[:, base + i * W: base + i * W + NW],
                        start=True, stop=True,
                    )
                S2 = psum_pool.tile([M, 2, W], f32, tag="S2", name=f"S2_{t}{p}")
                for i in range(2):
                    nc.tensor.matmul(
                        S2[:, i, 0:NW],
                        stat_sb[:, yoff: yoff + M],
                        Tvp[:, base + i * W: base + i * W + NW],
                        start=True, stop=True,
                    )
                S2s.append(S2)
                pp = slice(2 * p, 2 * p + 2)
                nc.scalar.square(s1[:, pp, 1: 1 + NW], S1[:, :, 0:NW])
                nc.scalar.square(s2[:, pp, 1: 1 + NW], S2[:, :, 0:NW])
            return (s1, s2, ss, None, M, out[o0: o0 + M])

        def stage_b(s1, s2, ss, S2s, M, out_ap):
            # per-pair drain chain keeps the kernel tail short: only the
            # last pair's work serializes after the final matmul
            mag = work_pool.tile([M, N_PER_CORE, W], f16, tag="mag",
                                 name="mag", bufs=4)
            for p in range(2):
                pp = slice(2 * p, 2 * p + 2)
                if S2s is not None:
                    # last super: fused sq(S2)+s1 on the DVE straight from
                    # PSUM (safe only here -- no later super reuses these
                    # PSUM buffers); trims ~2 us of ACT off the kernel tail
                    nc.vector._custom_dve(
                        sq_plus,
                        out=ss[:, pp, 1: 1 + NW],
                        in0=S2s[p][:, :, 0:NW],
                        in1=s1[:, pp, 1: 1 + NW],
                    )
                else:
                    nc.vector.tensor_add(
                        ss[:, pp, 1: 1 + NW], s1[:, pp, 1: 1 + NW],
                        s2[:, pp, 1: 1 + NW]
                    )
                # x-edge pad: out cols {0,511} dup cols {1,510}
                nc.scalar.copy(ss[:, pp, 0: W: W - 1],
                               ss[:, pp, 1: W - 1: W - 3])
                nc.scalar.sqrt(mag[:, pp], ss[:, pp])
                # store this image pair immediately, split into row-chunks
                # alternating rings: a LONE one-DMA store was measured to
                # dump all its descriptors on ONE DMA engine (~24 GB/s);
                # chunks spread across engines, per-pair stores start the
                # final drain earlier, and DMA_DIRECT2D issue costs ~0.6 us
                # of ring-queue time regardless of size, so splitting the
                # issue across both rings halves the tail's issue wall.
                CH = 32
                for k, r0 in enumerate(range(0, M, CH)):
                    r1 = min(r0 + CH, M)
                    ring = nc.sync if k % 2 == 0 else nc.scalar
                    ring.dma_start(out=out_ap[r0:r1, pp],
                                   in_=mag[r0:r1, pp])

        def stage_a_mini():
            Tvm, Dm = xprep("m", 32, W, W + 16, mit[:, 0], mit[:, 1],
                            mit[:, 2])
            S1 = psum_pool.tile([MINI_M, 2, W], f32, tag="S1", name="mS1")
            nc.tensor.matmul(S1[:, 0, 0:NW], statm_sb[:, 0:MINI_M],
                             Dm[:, 0:NW], start=True, stop=True)
            S2 = psum_pool.tile([MINI_M, 2, W], f32, tag="S2", name="mS2")
            nc.tensor.matmul(S2[:, 0, 0:NW], statm_sb[:, MINI_M: 2 * MINI_M],
                             Tvm[:, 0:NW], start=True, stop=True)
            s1 = work_pool.tile([MINI_M, W], f16, tag="ms1", bufs=1,
                                name="ms1")
            nc.scalar.square(s1[:, 1: 1 + NW], S1[:, 0, 0:NW])
            ss = work_pool.tile([MINI_M, W], f16, tag="mss", bufs=1,
                                name="mss")
            nc.vector._custom_dve(
                sq_plus, out=ss[:, 1: 1 + NW], in0=S2[:, 0, 0:NW],
                in1=s1[:, 1: 1 + NW],
            )
            return ss

        def stage_b_mini(ss):
            nc.scalar.copy(ss[:, 0: W: W - 1], ss[:, 1: W - 1: W - 3])
            mag = work_pool.tile([MINI_M, W], f16, tag="mmag", bufs=1,
                                 name="mmag")
            nc.scalar.sqrt(mag, ss)
            for n in range(N_PER_CORE):
                ring = nc.sync if n % 2 == 0 else nc.scalar
                ring.dma_start(out=out[MINI_O0:H, n],
                               in_=mag[7 * n: 7 * n + 7])

        # software pipeline, two units deep (as v1): emit stage_b(k-2)
        # after stage_a(k) so per-engine queues never head-of-line block.
        from collections import deque

        pending = deque()
        units = [0, "mini", 1, 2, 3]
        for unit in units:
            pending.append(
                (stage_a_mini(), "mini") if unit == "mini"
                else (stage_a(unit), unit)
            )
            if len(pending) > 2:
                args, kind = pending.popleft()
                stage_b_mini(args) if kind == "mini" else stage_b(*args)
        while pending:
            args, kind = pending.popleft()
            stage_b_mini(args) if kind == "mini" else stage_b(*args)


def _build_program_fast():
    import concourse.bacc as bacc
    import concourse.mybir as mybir
    import concourse.tile as tile

    sq_plus = _register_sq_plus()
    nc = bacc.Bacc(
        "TRN2", target_bir_lowering=False, debug=False, num_devices=N_CORES
    )
    img = nc.dram_tensor(
        "img", [3, H, N_PER_CORE, W], mybir.dt.float16, kind="ExternalInput"
    ).ap()
    stat = nc.dram_tensor(
        "stat", [128, 4 * 128], mybir.dt.float16, kind="ExternalInput"
    ).ap()
    statm = nc.dram_tensor(
        "statm", [32, 2 * MINI_M], mybir.dt.float16, kind="ExternalInput"
    ).ap()
    out = nc.dram_tensor(
        "out", [H, N_PER_CORE, W], mybir.dt.float16, kind="ExternalOutput"
    ).ap()
    with tile.TileContext(nc) as tc:
        _sobel_body_fast(tc, out, img, stat, statm, sq_plus)
    nc.compile()
    return nc


# ---------------------------------------------------------------------------
# Generic fallback (arbitrary kx/ky): full 2D conv as 18 banded fp32r
# matmuls per tile. Unchanged from v1.
# ---------------------------------------------------------------------------


def _build_stationaries(kx: np.ndarray, ky: np.ndarray):
    ks = (np.asarray(kx, np.float32), np.asarray(ky, np.float32))
    stat = np.zeros((18, TILE_K, TILE_M), np.float32)
    mini = np.zeros((18, G_MINI_K, G_MINI_M), np.float32)
    m = np.arange(TILE_M)
    mm = np.arange(6)
    i = 0
    for g in range(2):
        for c in range(3):
            for dx in range(3):
                for dy in range(3):
                    stat[i, m + dy, m] = ks[g][0, c, dy, dx]
                    for j in range(N_PER_CORE):
                        mini[i, j * 8 + mm + dy, j * 6 + mm] = ks[g][0, c, dy, dx]
                i += 1
    return (
        np.ascontiguousarray(stat.transpose(1, 0, 2)),
        np.ascontiguousarray(mini.transpose(1, 0, 2)),
    )


def _epilogue(nc, work_pool, psx, psy, rows, f32):
    s = work_pool.tile([rows, W], f32, tag="s", name="s")
    s2 = work_pool.tile([rows, NW], f32, tag="s2", name="s2")
    nc.scalar.square(s[:, 1: 1 + NW], psx)
    nc.scalar.square(s2, psy)
    nc.vector.tensor_add(s[:, 1: 1 + NW], s[:, 1: 1 + NW], s2)
    nc.vector.tensor_copy(s[:, 0:1], s[:, 1:2])
    nc.vector.tensor_copy(s[:, W - 1: W], s[:, W - 2: W - 1])
    mag = work_pool.tile([rows, W], f32, tag="mag", name="mag")
    nc.scalar.sqrt(mag, s)
    return mag


def _sobel_body(tc, out, img, stat_dram, stat_mini_dram):
    import concourse.mybir as mybir

    nc = tc.nc
    f32 = mybir.dt.float32
    mm_dt = mybir.dt.float32r

    img_yx = img.rearrange("n c y x -> n y c x")

    with (
        tc.tile_pool(name="const", bufs=1) as const_pool,
        tc.tile_pool(name="imgs", bufs=3) as img_pool,
        tc.tile_pool(name="work", bufs=4) as work_pool,
        tc.tile_pool(name="psum", bufs=2, space="PSUM") as psum_pool,
    ):
        stat_mini_sb = const_pool.tile([G_MINI_K, 18, G_MINI_M], mm_dt)
        nc.sync.dma_start(out=stat_mini_sb, in_=stat_mini_dram)
        mit = img_pool.tile([G_MINI_K, 3, W], mm_dt, tag="mit", bufs=1)
        for c in range(3):
            nc.sync.dma_start(out=mit[:, c, :], in_=img_yx[:, H - 8: H, c])
        stat_sb = const_pool.tile([TILE_K, 18, TILE_M], mm_dt)
        for j in range(5):
            nc.sync.dma_start(
                out=stat_sb[:, 2 * j: 2 * j + 2], in_=stat_dram[:, 2 * j: 2 * j + 2]
            )
        for j in range(5, 9):
            nc.scalar.dma_start(
                out=stat_sb[:, 2 * j: 2 * j + 2], in_=stat_dram[:, 2 * j: 2 * j + 2]
            )

        def big_tile(n, t):
            y0 = t * TILE_M
            its = []
            for c in range(3):
                itc = img_pool.tile(
                    [TILE_K, W], mm_dt, tag=f"it{c}", name=f"it{c}", bufs=6
                )
                nc.sync.dma_start(out=itc, in_=img_yx[n, y0: y0 + TILE_K, c])
                its.append(itc)

            psx = psum_pool.tile([TILE_M, NW], f32, tag="psx", name="psx")
            psy = psum_pool.tile([TILE_M, NW], f32, tag="psy", name="psy")
            for g, ps in ((0, psx), (1, psy)):
                mmi = 0
                for c in range(3):
                    for dx in range(3):
                        i = (g * 3 + c) * 3 + dx
                        nc.tensor.matmul(
                            ps,
                            stat_sb[:, i, :],
                            its[c][:, dx: dx + NW],
                            start=(mmi == 0),
                            stop=(mmi == 8),
                        )
                        mmi += 1

            mag = _epilogue(nc, work_pool, psx, psy, TILE_M, f32)
            nc.scalar.dma_start(out=out[n, 1 + y0: 1 + y0 + TILE_M, :], in_=mag)
            if t == 0:
                nc.scalar.dma_start(out=out[n, 0:1, :], in_=mag[0:1, :])

        def mini_tile():
            mpsx = psum_pool.tile([G_MINI_M, NW], f32, tag="mpsx", bufs=1,
                                  name="mpsx")
            mpsy = psum_pool.tile([G_MINI_M, NW], f32, tag="mpsy", bufs=1,
                                  name="mpsy")
            for g, ps in ((0, mpsx), (1, mpsy)):
                mmi = 0
                for c in range(3):
                    for dx in range(3):
                        i = (g * 3 + c) * 3 + dx
                        nc.tensor.matmul(
                            ps,
                            stat_mini_sb[:, i, :],
                            mit[:, c, dx: dx + NW],
                            start=(mmi == 0),
                            stop=(mmi == 8),
                        )
                        mmi += 1
            mmag = _epilogue(nc, work_pool, mpsx, mpsy, G_MINI_M, f32)
            for n in range(N_PER_CORE):
                nc.scalar.dma_start(
                    out=out[n, H - 7: H - 1, :], in_=mmag[n * 6: n * 6 + 6]
                )
                nc.scalar.dma_start(
                    out=out[n, H - 1: H, :], in_=mmag[n * 6 + 5: n * 6 + 6]
                )

        mini_tile()
        for n in range(N_PER_CORE):
            for t in range(N_TILES):
                big_tile(n, t)


def _build_program():
    import concourse.bacc as bacc
    import concourse.mybir as mybir
    import concourse.tile as tile

    nc = bacc.Bacc(
        "TRN2",
        target_bir_lowering=False,
        debug=False,
        num_devices=N_CORES,
    )
    img = nc.dram_tensor(
        "img", [N_PER_CORE, 3, H, W], mybir.dt.float32r, kind="ExternalInput"
    ).ap()
    stat = nc.dram_tensor(
        "stat", [TILE_K, 18, TILE_M], mybir.dt.float32r, kind="ExternalInput"
    ).ap()
    stat_mini = nc.dram_tensor(
        "stat_mini", [G_MINI_K, 18, G_MINI_M], mybir.dt.float32r,
        kind="ExternalInput"
    ).ap()
    out = nc.dram_tensor(
        "out", [N_PER_CORE, H, W], mybir.dt.float32, kind="ExternalOutput"
    ).ap()

    with tile.TileContext(nc) as tc:
        _sobel_body(tc, out, img, stat, stat_mini)
    nc.compile()
    return nc


def _run(nc, in_maps):
    global LAST_RESULTS
    from concourse.bass_utils import run_bass_kernel_spmd

    trace = os.environ.get("SOBEL_TRACE", "0") == "1"
    res = run_bass_kernel_spmd(
        nc, in_maps, core_ids=list(range(N_CORES)), trace=trace
    )
    LAST_RESULTS = res
    return np.concatenate([res.results[c]["out"] for c in range(N_CORES)], axis=0)


def kernel(img: np.ndarray, kx: np.ndarray, ky: np.ndarray) -> np.ndarray:
    img = np.ascontiguousarray(np.asarray(img, dtype=np.float32))
    assert img.shape == (N_FULL, 3, H, W), img.shape

    fast = (
        _try_fast(kx, ky)
        if os.environ.get("SOBEL_NO_FAST", "0") != "1"
        else None
    )
    if fast is not None:
        stat, statm, key = fast
        key = key + (os.environ.get("SOBEL_DVE_DRAIN", "2"),)
        if key not in _CACHE:
            _CACHE[key] = _build_program_fast()
        nc = _CACHE[key]
        img16 = img.astype(np.float16)
        in_maps = [
            {
                "img": np.ascontiguousarray(
                    img16[c * N_PER_CORE: (c + 1) * N_PER_CORE].transpose(
                        1, 2, 0, 3
                    )
                ),
                "stat": stat,
                "statm": statm,
            }
            for c in range(N_CORES)
        ]
        out = _run(nc, in_maps)  # [8 cores x [512, 4, 512]] fp16, y-major
        out = out.reshape(N_CORES, H, N_PER_CORE, W).transpose(0, 2, 1, 3)
        return (
            np.ascontiguousarray(out)
            .reshape(N_FULL, 1, H, W)
            .astype(np.float32)
        )

    stat, stat_mini = _build_stationaries(kx, ky)
    if "gen" not in _CACHE:
        _CACHE["gen"] = _build_program()
    nc = _CACHE["gen"]
    in_maps = [
        {
            "img": img[c * N_PER_CORE: (c + 1) * N_PER_CORE],
            "stat": stat,
            "stat_mini": stat_mini,
        }
        for c in range(N_CORES)
    ]
    out = _run(nc, in_maps)
    return out.reshape(N_FULL, 1, H, W)
